# revision 45
# baseline (speedup 1.0000x reference)
"""MultiHeadAttention (B=4, S=2048, D=1024, H=16, causal + key mask) on 8 trn2 cores.

Sharding: batch x head-group. Core (b, hg) owns batch b and 8 heads (4 pairs
of 2). Host sums the two half-partials per batch and adds bp + bv@Wp (the V
bias is equivalent to a constant output offset because softmax rows sum to 1).

Per-core kernel (bf16 data path, fp32 PSUM/softmax stats):
  - x^T for its batch streamed in 4 chunks of 512 rows; projections produce
    Q^T/K^T [feat=128 (pair-of-heads), rows] per pair (W slice as lhsT).
    Q/K biases folded into the PSUM->SBUF evacuation (DVE tensor_scalar).
  - V transposed into [row, hd] layout via PE transpose into a 192-wide slot
    ([V_h0 | ones | zeros | V_h1]). The shared ones column makes both PV
    matmuls accumulate the softmax denominator: h0 uses lhsT cols [0:65]
    (denom at psum row 64), h1 uses cols [32:160] (denom at row 32, data on
    partitions 64-127 so no cross-partition move is ever needed).
  - Scores computed transposed: S^T[k, q] = K^T.T @ Q^T (K=64 -> the two
    heads' matmuls auto-pack as 64x128 row tiles and run concurrently).
    Diagonal blocks restrict N to the causally-live q range.
  - exp on ScalarE (key mask as per-partition bias), output bf16, emitted
    before the mask so it never waits on DVE; the causal triangle of the
    diagonal square is then zeroed multiplicatively on bf16 (cheap 2x mode).
  - Denominator rows of all 4 pairs are DMA-gathered into one [8, 512] tile
    (single-partition moves are DMA-exempt from the partition-alignment
    rule); ONE batched DVE reciprocal per group (free-dim bound, so 8 rows
    cost the same as 1); the reciprocals are broadcast to 128 partitions
    with a K=8 selection matmul into PSUM (broadcast DMAs cost ~1us per
    single-partition-source descriptor and saturated all 16 DMA engines).
  - Output projection accumulates the 4 pairs in PSUM; evacuation on DVE in
    bf16; partial outputs DMA'd to HBM in bf16. Outproj lags one group so
    the normalize chain latency stays off the critical path.
  - Projection chunk c+1 is emitted interleaved with attention group c
    (group g only needs chunks <= g), so projection PE/DVE work overlaps
    the ScalarE-bound attention stretches.
"""

import os

import numpy as np

DBG = bool(int(os.environ.get("MHADBG", "0")))

P = 128
B, S, D, H = 4, 2048, 1024, 16
HD = D // H          # 64
NCORES = 8
NP = 4               # head pairs per core (8 heads)
NC = S // 512        # 4 row chunks
NB = S // P          # 16 key blocks
KD = D // P          # 8 contraction chunks
FPC = NP * P         # 512 features per core
VW = 192             # V slot: [0:64]=V_h0, [64]=ones, [65:96]=0, [96:160]=V_h1

_CACHE = {}


def _build_nc():
    import concourse.mybir as mybir
    from concourse import bacc
    from concourse.tile import TileContext
    from concourse.masks import make_identity
    from contextlib import ExitStack

    f32 = mybir.dt.float32
    f32r = mybir.dt.float32r
    bf16 = mybir.dt.bfloat16
    AF = mybir.ActivationFunctionType

    nc = bacc.Bacc("TRN2", target_bir_lowering=False, debug=False,
                   num_devices=NCORES)

    xT_d = nc.dram_tensor("xT", [P, NC, KD, 512], bf16,
                          kind="ExternalInput").ap()
    wq_d = nc.dram_tensor("wq", [P, KD, FPC], bf16, kind="ExternalInput").ap()
    wk_d = nc.dram_tensor("wk", [P, KD, FPC], bf16, kind="ExternalInput").ap()
    wv_d = nc.dram_tensor("wv", [P, KD, FPC], bf16, kind="ExternalInput").ap()
    bq_d = nc.dram_tensor("bq", [P, NP], f32, kind="ExternalInput").ap()
    bk_d = nc.dram_tensor("bk", [P, NP], f32, kind="ExternalInput").ap()
    wp_d = nc.dram_tensor("wp", [P, NP, D], bf16, kind="ExternalInput").ap()
    mb_d = nc.dram_tensor("maskb", [P, NB], f32, kind="ExternalInput").ap()
    cm_d = nc.dram_tensor("cmask", [P, 2, P], bf16,
                          kind="ExternalInput").ap()
    es_d = nc.dram_tensor("esel", [8, NP, P], f32r,
                          kind="ExternalInput").ap()
    yp_d = nc.dram_tensor("yp", [S, D], bf16, kind="ExternalOutput").ap()
    if DBG:
        dbg_qt = nc.dram_tensor("dbg_qt", [P, NC, NP, 512], bf16,
                                kind="ExternalOutput").ap()
        dbg_kt = nc.dram_tensor("dbg_kt", [P, NC, NP, 512], bf16,
                                kind="ExternalOutput").ap()
        dbg_v = nc.dram_tensor("dbg_v", [P, NC, NP, 4, VW], bf16,
                               kind="ExternalOutput").ap()
        dbg_at = nc.dram_tensor("dbg_at", [P, NC, NP, 512], bf16,
                                kind="ExternalOutput").ap()

    with TileContext(nc) as tc:
        with ExitStack() as ctx:
            consts = ctx.enter_context(tc.tile_pool(name="consts", bufs=1))
            big = ctx.enter_context(tc.tile_pool(name="big", bufs=1))
            xpool = ctx.enter_context(tc.tile_pool(name="xpool", bufs=2))
            vtpool = ctx.enter_context(tc.tile_pool(name="vtpool", bufs=2))
            ptpool = ctx.enter_context(tc.tile_pool(name="ptpool", bufs=4))
            npool = ctx.enter_context(tc.tile_pool(name="npool", bufs=2))
            pvsbpool = ctx.enter_context(
                tc.tile_pool(name="pvsbpool", bufs=18))
            ypool = ctx.enter_context(tc.tile_pool(name="ypool", bufs=3))
            pspool = ctx.enter_context(
                tc.tile_pool(name="pspool", bufs=2, space="PSUM"))
            sc2pool = ctx.enter_context(
                tc.tile_pool(name="sc2pool", bufs=2, space="PSUM"))
            pvpool = ctx.enter_context(
                tc.tile_pool(name="pvpool", bufs=2, space="PSUM"))

            # ---- constants (xt0 first: the first projection needs it) ----
            xt0 = xpool.tile([P, KD, 512], bf16, tag="xt", name="xt0")
            nc.sync.dma_start(xt0[:], xT_d[:, 0])
            wq_sb = consts.tile([P, KD, FPC], bf16, tag="wq")
            wk_sb = consts.tile([P, KD, FPC], bf16, tag="wk")
            wv_sb = consts.tile([P, KD, FPC], bf16, tag="wv")
            wp_sb = consts.tile([P, NP, D], bf16, tag="wp")
            nc.sync.dma_start(wk_sb[:], wk_d)
            nc.sync.dma_start(wq_sb[:], wq_d)
            nc.sync.dma_start(wv_sb[:], wv_d)
            bq_sb = consts.tile([P, NP], f32, tag="bq")
            bk_sb = consts.tile([P, NP], f32, tag="bk")
            nc.sync.dma_start(bq_sb[:], bq_d)
            nc.sync.dma_start(bk_sb[:], bk_d)
            mb_sb = consts.tile([P, NB], f32, tag="mb")
            nc.sync.dma_start(mb_sb[:], mb_d)
            cm_sb = consts.tile([P, 2, P], bf16, tag="cm")
            nc.sync.dma_start(cm_sb[:], cm_d)
            ident = consts.tile([P, P], f32, tag="ident")
            make_identity(nc, ident[:])

            # ---- per-chunk activations (distinct tiles -> clean deps) ----
            qt_c = [big.tile([P, NP, 512], bf16, tag=f"qt{c}",
                             name=f"qt{c}") for c in range(NC)]
            kt_c = [big.tile([P, NP, 512], bf16, tag=f"kt{c}",
                             name=f"kt{c}") for c in range(NC)]
            v_c = [big.tile([P, NP, 4, VW], bf16, tag=f"v{c}",
                            name=f"v{c}") for c in range(NC)]
            at_gp = [[big.tile([P, 512], bf16, tag=f"at{g}_{p}",
                               name=f"at{g}_{p}") for p in range(NP)]
                     for g in range(NC)]
            for c in range(NC):
                nc.vector.memset(v_c[c][:, :, :, HD], 1.0)
                nc.vector.memset(v_c[c][:, :, :, HD + 1:96], 0.0)
            es_sb = consts.tile([8, NP, P], f32r, tag="esel")
            nc.sync.dma_start(es_sb[:], es_d)
            nc.sync.dma_start(wp_sb[:], wp_d)

            def proj_part(c, which, xt):
                w_sb = (wq_sb, wk_sb, wv_sb)[which]
                for mt in range(NP):
                    if True:
                        ps = pspool.tile([P, 512], f32, tag="ps",
                                         name=f"pj_{c}_{which}_{mt}")
                        for o in range(KD):
                            nc.tensor.matmul(
                                ps[:], lhsT=w_sb[:, o, mt * P:(mt + 1) * P],
                                rhs=xt[:, o, :],
                                start=(o == 0), stop=(o == KD - 1))
                        if which == 0:
                            nc.vector.tensor_scalar_add(
                                qt_c[c][:, mt, :], ps[:],
                                bq_sb[:, mt:mt + 1])
                        elif which == 1:
                            nc.vector.tensor_scalar_add(
                                kt_c[c][:, mt, :], ps[:],
                                bk_sb[:, mt:mt + 1])
                        else:
                            vt = vtpool.tile([P, 512], f32, tag="vt")
                            nc.vector.tensor_copy(vt[:], ps[:])
                            for t in range(4):
                                trp = pspool.tile([P, P], f32, tag="ps",
                                                  name=f"tr_{c}_{mt}_{t}")
                                nc.tensor.transpose(
                                    trp[:], vt[:, t * P:(t + 1) * P],
                                    ident[:])
                                dst = (v_c[c][:, mt, t, :]
                                       .rearrange("p (h x) -> p h x", h=2)
                                       [:, :, 0:HD])
                                src = trp[:].rearrange(
                                    "p (h x) -> p h x", h=2)
                                nc.vector.tensor_copy(dst, src)

            def attention(pair, g, pvsb_g, dn_g):
                nkb = 4 * (g + 1)
                pvs = [pvpool.tile([P, 512], f32, tag="pv",
                                   name=f"pv_{pair}_{g}_{h}")
                       for h in range(2)]
                for kb in range(nkb):
                    j = kb - 4 * g
                    q0 = P * j if j >= 0 else 0
                    sc2 = sc2pool.tile([P, 1024], f32, tag="sc2",
                                       name=f"sc2_{pair}_{g}_{kb}")
                    for h in range(2):
                        hsl = slice(HD * h, HD * (h + 1))
                        nc.tensor.matmul(
                            sc2[:, h * 512 + q0:(h + 1) * 512],
                            lhsT=kt_c[kb // 4][hsl, pair,
                                               (kb % 4) * P:(kb % 4 + 1) * P],
                            rhs=qt_c[g][hsl, pair, q0:512],
                            start=True, stop=True)
                    sc2r = sc2[:].rearrange("p (h q) -> p h q", h=2)
                    pt = ptpool.tile([P, 2, 512], bf16, tag="pt")
                    mbc = mb_sb[:, kb:kb + 1]
                    if j >= 0:
                        # exp first (keeps ScalarE off the DVE critical
                        # path), then zero the causal triangle of the
                        # diagonal square multiplicatively on bf16
                        nc.scalar.activation(pt[:, :, q0:512],
                                             sc2r[:, :, q0:512],
                                             AF.Exp, bias=mbc)
                        sq = pt[:, :, q0:q0 + P]
                        nc.vector.tensor_mul(sq, sq, cm_sb[:])
                    else:
                        nc.scalar.activation(pt[:], sc2r, AF.Exp, bias=mbc)
                    vb = v_c[kb // 4][:, pair, kb % 4, :]
                    # h0: lhsT cols [0:65] -> rows 0-63 data, row 64 denom.
                    # h1: lhsT cols [32:160] -> row 32 denom (the shared ones
                    #     column), rows 64-127 data (partition-aligned with
                    #     at_gp's head-1 half: no cross-partition move).
                    nc.tensor.matmul(
                        pvs[0][0:HD + 1, q0:512], lhsT=vb[:, 0:HD + 1],
                        rhs=pt[:, 0, q0:512],
                        start=(kb == 0), stop=(kb == nkb - 1))
                    nc.tensor.matmul(
                        pvs[1][:, q0:512], lhsT=vb[:, 32:160],
                        rhs=pt[:, 1, q0:512],
                        start=(kb == 0), stop=(kb == nkb - 1))
                for h in range(2):
                    r = 2 * pair + h
                    pvsb = pvsbpool.tile([P, 512], f32, tag="pvsb",
                                         name=f"pvsb_{pair}_{g}_{h}")
                    dr = HD if h == 0 else 32          # denominator row
                    if h == 0:
                        nc.vector.tensor_copy(pvsb[0:HD + 1, :],
                                              pvs[h][0:HD + 1, :])
                    else:
                        nc.vector.tensor_copy(pvsb[:], pvs[h][:])
                    # gather the denominator row (partition move via DMA)
                    nc.sync.dma_start(dn_g[r:r + 1, :], pvsb[dr:dr + 1, :])
                    pvsb_g[r] = pvsb

            def normalize(g, pvsb_g, dn_g):
                rcg = npool.tile([8, 512], f32r, tag="rcg")
                with nc.allow_low_precision(reason="f32r broadcast matmul"):
                    nc.vector.reciprocal(rcg[:], dn_g[:])
                for pair in range(NP):
                    # broadcast 1/denom to the pair's 128 partitions with a
                    # K=8 selection matmul (avoids slow single-partition-
                    # source broadcast DMAs entirely)
                    sxp = pspool.tile([P, 512], f32, tag="ps",
                                      name=f"sx_{g}_{pair}")
                    nc.tensor.matmul(
                        sxp[:], lhsT=es_sb[:, pair, :],
                        rhs=rcg[:], start=True, stop=True)
                    nc.vector.tensor_mul(
                        at_gp[g][pair][0:HD, :],
                        pvsb_g[2 * pair][0:HD, :], sxp[0:HD, :])
                    nc.vector.tensor_mul(
                        at_gp[g][pair][HD:2 * HD, :],
                        pvsb_g[2 * pair + 1][HD:2 * HD, :],
                        sxp[HD:2 * HD, :])

            def outproj(g):
                for qi in range(4):
                    q0 = g * 512 + qi * P
                    yb = ypool.tile([P, D], bf16, tag="yb")
                    for half in range(2):
                        ps = pspool.tile([P, 512], f32, tag="ps",
                                         name=f"yps_{g}_{qi}_{half}")
                        for pair in range(NP):
                            nc.tensor.matmul(
                                ps[:],
                                lhsT=at_gp[g][pair][:, qi * P:(qi + 1) * P],
                                rhs=wp_sb[:, pair,
                                          half * 512:(half + 1) * 512],
                                start=(pair == 0), stop=(pair == NP - 1))
                        nc.vector.tensor_copy(
                            yb[:, half * 512:(half + 1) * 512], ps[:])
                    nc.sync.dma_start(yp_d[q0:q0 + P, :], yb[:])

            # chunk 0 projection (head of the pipeline): K, Q first so the
            # first scores/exps start as early as possible
            for w in (1, 0, 2):
                proj_part(0, w, xt0)
            # prefetch the next x chunk one iteration ahead (xpool bufs=2)
            xts = {1: xpool.tile([P, KD, 512], bf16, tag="xt", name="xt1")}
            nc.sync.dma_start(xts[1][:], xT_d[:, 1])
            # steady state: attention group g = c-1 overlaps projection of
            # chunk c (group g only needs chunks <= g); attention pairs are
            # emitted before proj parts so scores never queue behind proj
            # matmuls in the in-order PE stream
            for c in range(1, NC + 1):
                g = c - 1
                pvsb_g = {}
                dn_g = npool.tile([8, 512], f32, tag="dn", name=f"dn{g}")
                for pair in range(NP):
                    attention(pair, g, pvsb_g, dn_g)
                    if pair == 0 and g >= 1:
                        outproj(g - 1)
                    if c < NC and pair < 3:
                        proj_part(c, pair, xts[c])
                if c + 1 < NC:
                    xts[c + 1] = xpool.tile([P, KD, 512], bf16, tag="xt",
                                            name=f"xt{c + 1}")
                    nc.sync.dma_start(xts[c + 1][:], xT_d[:, c + 1])
                normalize(g, pvsb_g, dn_g)
            outproj(NC - 1)

            if DBG:
                for c in range(NC):
                    nc.sync.dma_start(dbg_qt[:, c], qt_c[c][:])
                    nc.sync.dma_start(dbg_kt[:, c], kt_c[c][:])
                    nc.sync.dma_start(dbg_v[:, c], v_c[c][:])
                    nc.sync.dma_start(dbg_at[:, c], at_g[c][:])

    nc.compile()
    return nc


def _get_nc():
    if "nc" not in _CACHE:
        _CACHE["nc"] = _build_nc()
    return _CACHE["nc"]


def make_in_maps(x, attention_mask, Wq, bq, Wk, bk, Wv, bv, Wp, bp):
    """Host-side sharding: 8 per-core input maps (core = b * 2 + hg)."""
    import ml_dtypes
    bf = ml_dtypes.bfloat16
    x = np.asarray(x, dtype=np.float32)
    scale = np.float32(1.0 / np.sqrt(HD))
    Wq = np.asarray(Wq, np.float32) * scale
    bq = np.asarray(bq, np.float32) * scale
    Wk = np.asarray(Wk, np.float32)
    bk = np.asarray(bk, np.float32)
    Wv = np.asarray(Wv, np.float32)
    Wp = np.asarray(Wp, np.float32)
    mask = np.asarray(attention_mask).astype(np.float32)

    # causal triangle for the diagonal 128x128 square (j-invariant):
    # row p (key), col q: valid iff p <= q.
    pp = np.arange(P)[:, None]
    qq = np.arange(P)[None, :]
    tri = np.where(pp <= qq, 1.0, 0.0).astype(np.float32)
    cm = np.ascontiguousarray(
        np.broadcast_to(tri[:, None, :], (P, 2, P)).astype(ml_dtypes.bfloat16))

    # selection matrix for the reciprocal-broadcast matmul:
    # esel[r, pair, m] = 1 iff r == 2*pair + m//64
    es = np.zeros((8, NP, P), dtype=np.float32)
    for pr in range(NP):
        es[2 * pr, pr, 0:HD] = 1.0
        es[2 * pr + 1, pr, HD:P] = 1.0

    in_maps = []
    for core in range(NCORES):
        b, hg = divmod(core, 2)
        fsl = slice(hg * FPC, (hg + 1) * FPC)
        xT = x[b].T.reshape(KD, P, NC, 512).transpose(1, 2, 0, 3)
        mb = ((mask[b] - 1.0) * np.float32(1e9)).reshape(NB, P).T
        in_maps.append({
            "xT": np.ascontiguousarray(xT.astype(bf)),
            "wq": np.ascontiguousarray(
                Wq[:, fsl].reshape(KD, P, FPC).transpose(1, 0, 2).astype(bf)),
            "wk": np.ascontiguousarray(
                Wk[:, fsl].reshape(KD, P, FPC).transpose(1, 0, 2).astype(bf)),
            "wv": np.ascontiguousarray(
                Wv[:, fsl].reshape(KD, P, FPC).transpose(1, 0, 2).astype(bf)),
            "bq": np.ascontiguousarray(bq[fsl].reshape(NP, P).T),
            "bk": np.ascontiguousarray(bk[fsl].reshape(NP, P).T),
            "wp": np.ascontiguousarray(
                Wp[fsl, :].reshape(NP, P, D).transpose(1, 0, 2).astype(bf)),
            "maskb": np.ascontiguousarray(mb),
            "cmask": cm,
            "esel": es,
        })
    return in_maps


def run(inputs, trace=False, tmpdir=None):
    """Compile (cached) + run on 8 cores. Returns (output, BassKernelResults)."""
    from concourse import bass_utils
    nc = _get_nc()
    in_maps = make_in_maps(**inputs)
    kwargs = {}
    if trace:
        kwargs = dict(trace=True, tmpdir=tmpdir)
    res = bass_utils.run_bass_kernel_spmd(
        nc, in_maps, core_ids=list(range(NCORES)), **kwargs)
    bv = np.asarray(inputs["bv"], np.float64)
    Wp = np.asarray(inputs["Wp"], np.float64)
    bp = np.asarray(inputs["bp"], np.float64)
    const = bv @ Wp + bp  # V-bias contribution (softmax rows sum to 1) + bp
    out = np.empty((B, S, D), dtype=np.float32)
    for b in range(B):
        acc = (res.results[2 * b]["yp"].astype(np.float64)
               + res.results[2 * b + 1]["yp"].astype(np.float64) + const)
        out[b] = acc.astype(np.float32)
    return out, res


def kernel(**inputs) -> np.ndarray:
    out, _ = run(inputs, trace=False)
    return out


# revision 46
# speedup vs baseline: 1.1867x; 1.1867x over previous
"""MultiHeadAttention (B=4, S=2048, D=1024, H=16, causal + key mask) on 8 trn2 cores.

Sharding: batch x head-group. Core (b, hg) owns batch b and 8 heads (4 pairs
of 2). Host sums the two half-partials per batch and adds bp + bv@Wp (the V
bias is equivalent to a constant output offset because softmax rows sum to 1).

Per-core kernel (bf16 data path, fp32 PSUM/softmax stats):
  - x^T for its batch streamed in 4 chunks of 512 rows; projections produce
    Q^T/K^T [feat=128 (pair-of-heads), rows] per pair (W slice as lhsT).
    Q/K biases folded into the PSUM->SBUF evacuation (DVE tensor_scalar).
  - V transposed into [row, hd] layout via PE transpose into a 192-wide slot
    ([V_h0 | ones | zeros | V_h1]). The shared ones column makes both PV
    matmuls accumulate the softmax denominator: h0 uses lhsT cols [0:65]
    (denom at psum row 64), h1 uses cols [32:160] (denom at row 32, data on
    partitions 64-127 so no cross-partition move is ever needed).
  - Scores computed transposed: S^T[k, q] = K^T.T @ Q^T (K=64 -> the two
    heads' matmuls auto-pack as 64x128 row tiles and run concurrently).
    Diagonal blocks restrict N to the causally-live q range.
  - exp on ScalarE (key mask as per-partition bias), output bf16, emitted
    before the mask so it never waits on DVE; the causal triangle of the
    diagonal square is then zeroed multiplicatively on bf16 (cheap 2x mode).
  - Denominator rows of all 4 pairs are DMA-gathered into one [8, 512] tile
    (single-partition moves are DMA-exempt from the partition-alignment
    rule); ONE batched DVE reciprocal per group (free-dim bound, so 8 rows
    cost the same as 1); the reciprocals are broadcast to 128 partitions
    with a K=8 selection matmul into PSUM (broadcast DMAs cost ~1us per
    single-partition-source descriptor and saturated all 16 DMA engines).
  - Output projection accumulates the 4 pairs in PSUM; evacuation on DVE in
    bf16; partial outputs DMA'd to HBM in bf16. Outproj lags one group so
    the normalize chain latency stays off the critical path.
  - Projection chunk c+1 is emitted interleaved with attention group c
    (group g only needs chunks <= g), so projection PE/DVE work overlaps
    the ScalarE-bound attention stretches.
"""

import os

import numpy as np

DBG = bool(int(os.environ.get("MHADBG", "0")))

P = 128
B, S, D, H = 4, 2048, 1024, 16
HD = D // H          # 64
NCORES = 8
NP = 4               # head pairs per core (8 heads)
NC = S // 512        # 4 row chunks
NB = S // P          # 16 key blocks
KD = D // P          # 8 contraction chunks
FPC = NP * P         # 512 features per core
VW = 192             # V slot: [0:64]=V_h0, [64]=ones, [65:96]=0, [96:160]=V_h1

_CACHE = {}


def _build_nc():
    import concourse.mybir as mybir
    from concourse import bacc
    from concourse.tile import TileContext
    from concourse.masks import make_identity
    from contextlib import ExitStack

    f32 = mybir.dt.float32
    f32r = mybir.dt.float32r
    bf16 = mybir.dt.bfloat16
    AF = mybir.ActivationFunctionType

    nc = bacc.Bacc("TRN2", target_bir_lowering=False, debug=False,
                   num_devices=NCORES)

    xT_d = nc.dram_tensor("xT", [P, NC, KD, 512], bf16,
                          kind="ExternalInput").ap()
    wq_d = nc.dram_tensor("wq", [P, KD, FPC], bf16, kind="ExternalInput").ap()
    wk_d = nc.dram_tensor("wk", [P, KD, FPC], bf16, kind="ExternalInput").ap()
    wv_d = nc.dram_tensor("wv", [P, KD, FPC], bf16, kind="ExternalInput").ap()
    bq_d = nc.dram_tensor("bq", [P, NP], f32, kind="ExternalInput").ap()
    bk_d = nc.dram_tensor("bk", [P, NP], f32, kind="ExternalInput").ap()
    wp_d = nc.dram_tensor("wp", [P, NP, D], bf16, kind="ExternalInput").ap()
    mb_d = nc.dram_tensor("maskb", [P, NB], f32, kind="ExternalInput").ap()
    cm_d = nc.dram_tensor("cmask", [P, 2, P], bf16,
                          kind="ExternalInput").ap()
    es_d = nc.dram_tensor("esel", [8, NP, P], f32r,
                          kind="ExternalInput").ap()
    yp_d = nc.dram_tensor("yp", [S, D], bf16, kind="ExternalOutput").ap()
    if DBG:
        dbg_qt = nc.dram_tensor("dbg_qt", [P, NC, NP, 512], bf16,
                                kind="ExternalOutput").ap()
        dbg_kt = nc.dram_tensor("dbg_kt", [P, NC, NP, 512], bf16,
                                kind="ExternalOutput").ap()
        dbg_v = nc.dram_tensor("dbg_v", [P, NC, NP, 4, VW], bf16,
                               kind="ExternalOutput").ap()
        dbg_at = nc.dram_tensor("dbg_at", [P, NC, NP, 512], bf16,
                                kind="ExternalOutput").ap()

    with TileContext(nc) as tc:
        with ExitStack() as ctx:
            consts = ctx.enter_context(tc.tile_pool(name="consts", bufs=1))
            big = ctx.enter_context(tc.tile_pool(name="big", bufs=1))
            xpool = ctx.enter_context(tc.tile_pool(name="xpool", bufs=2))
            vtpool = ctx.enter_context(tc.tile_pool(name="vtpool", bufs=2))
            ptpool = ctx.enter_context(tc.tile_pool(name="ptpool", bufs=4))
            npool = ctx.enter_context(tc.tile_pool(name="npool", bufs=2))
            pvsbpool = ctx.enter_context(
                tc.tile_pool(name="pvsbpool", bufs=18))
            ypool = ctx.enter_context(tc.tile_pool(name="ypool", bufs=3))
            pspool = ctx.enter_context(
                tc.tile_pool(name="pspool", bufs=2, space="PSUM"))
            sc2pool = ctx.enter_context(
                tc.tile_pool(name="sc2pool", bufs=2, space="PSUM"))
            pvpool = ctx.enter_context(
                tc.tile_pool(name="pvpool", bufs=2, space="PSUM"))

            # ---- constants (xt0 first: the first projection needs it) ----
            xt0 = xpool.tile([P, KD, 512], bf16, tag="xt", name="xt0")
            nc.sync.dma_start(xt0[:], xT_d[:, 0])
            wq_sb = consts.tile([P, KD, FPC], bf16, tag="wq")
            wk_sb = consts.tile([P, KD, FPC], bf16, tag="wk")
            wv_sb = consts.tile([P, KD, FPC], bf16, tag="wv")
            wp_sb = consts.tile([P, NP, D], bf16, tag="wp")
            nc.sync.dma_start(wk_sb[:], wk_d)
            nc.sync.dma_start(wq_sb[:], wq_d)
            nc.sync.dma_start(wv_sb[:], wv_d)
            bq_sb = consts.tile([P, NP], f32, tag="bq")
            bk_sb = consts.tile([P, NP], f32, tag="bk")
            nc.sync.dma_start(bq_sb[:], bq_d)
            nc.sync.dma_start(bk_sb[:], bk_d)
            mb_sb = consts.tile([P, NB], f32, tag="mb")
            nc.sync.dma_start(mb_sb[:], mb_d)
            cm_sb = consts.tile([P, 2, P], bf16, tag="cm")
            nc.sync.dma_start(cm_sb[:], cm_d)
            ident = consts.tile([P, P], f32, tag="ident")
            make_identity(nc, ident[:])

            # ---- per-chunk activations (distinct tiles -> clean deps) ----
            qt_c = [big.tile([P, NP, 512], bf16, tag=f"qt{c}",
                             name=f"qt{c}") for c in range(NC)]
            kt_c = [big.tile([P, NP, 512], bf16, tag=f"kt{c}",
                             name=f"kt{c}") for c in range(NC)]
            v_c = [big.tile([P, NP, 4, VW], bf16, tag=f"v{c}",
                            name=f"v{c}") for c in range(NC)]
            at_gp = [[big.tile([P, 512], bf16, tag=f"at{g}_{p}",
                               name=f"at{g}_{p}") for p in range(NP)]
                     for g in range(NC)]
            for c in range(NC):
                nc.vector.memset(v_c[c][:, :, :, HD], 1.0)
                nc.vector.memset(v_c[c][:, :, :, HD + 1:96], 0.0)
            es_sb = consts.tile([8, NP, P], f32r, tag="esel")
            nc.sync.dma_start(es_sb[:], es_d)
            nc.sync.dma_start(wp_sb[:], wp_d)

            def proj_part(c, which, xt):
                w_sb = (wq_sb, wk_sb, wv_sb)[which]
                for mt in range(NP):
                    if True:
                        ps = pspool.tile([P, 512], f32, tag="ps",
                                         name=f"pj_{c}_{which}_{mt}")
                        for o in range(KD):
                            nc.tensor.matmul(
                                ps[:], lhsT=w_sb[:, o, mt * P:(mt + 1) * P],
                                rhs=xt[:, o, :],
                                start=(o == 0), stop=(o == KD - 1))
                        if which == 0:
                            nc.vector.tensor_scalar_add(
                                qt_c[c][:, mt, :], ps[:],
                                bq_sb[:, mt:mt + 1])
                        elif which == 1:
                            nc.vector.tensor_scalar_add(
                                kt_c[c][:, mt, :], ps[:],
                                bk_sb[:, mt:mt + 1])
                        else:
                            vt = vtpool.tile([P, 512], f32, tag="vt")
                            nc.vector.tensor_copy(vt[:], ps[:])
                            for t in range(4):
                                trp = pspool.tile([P, P], f32, tag="ps",
                                                  name=f"tr_{c}_{mt}_{t}")
                                nc.tensor.transpose(
                                    trp[:], vt[:, t * P:(t + 1) * P],
                                    ident[:])
                                dst = (v_c[c][:, mt, t, :]
                                       .rearrange("p (h x) -> p h x", h=2)
                                       [:, :, 0:HD])
                                src = trp[:].rearrange(
                                    "p (h x) -> p h x", h=2)
                                nc.vector.tensor_copy(dst, src)

            def attention(pair, g, pvsb_g, dn_g):
                nkb = 4 * (g + 1)
                pvs = [pvpool.tile([P, 512], f32, tag="pv",
                                   name=f"pv_{pair}_{g}_{h}")
                       for h in range(2)]
                for kb in range(nkb):
                    j = kb - 4 * g
                    q0 = P * j if j >= 0 else 0
                    sc2 = sc2pool.tile([P, 1024], f32, tag="sc2",
                                       name=f"sc2_{pair}_{g}_{kb}")
                    for h in range(2):
                        hsl = slice(HD * h, HD * (h + 1))
                        nc.tensor.matmul(
                            sc2[:, h * 512 + q0:(h + 1) * 512],
                            lhsT=kt_c[kb // 4][hsl, pair,
                                               (kb % 4) * P:(kb % 4 + 1) * P],
                            rhs=qt_c[g][hsl, pair, q0:512],
                            start=True, stop=True)
                    sc2r = sc2[:].rearrange("p (h q) -> p h q", h=2)
                    pt = ptpool.tile([P, 2, 512], bf16, tag="pt")
                    mbc = mb_sb[:, kb:kb + 1]
                    if j >= 0:
                        # exp first (keeps ScalarE off the DVE critical
                        # path), then zero the causal triangle of the
                        # diagonal square multiplicatively on bf16
                        nc.scalar.activation(pt[:, :, q0:512],
                                             sc2r[:, :, q0:512],
                                             AF.Exp, bias=mbc)
                        sq = pt[:, :, q0:q0 + P]
                        nc.vector.tensor_mul(sq, sq, cm_sb[:])
                    else:
                        nc.scalar.activation(pt[:], sc2r, AF.Exp, bias=mbc)
                    vb = v_c[kb // 4][:, pair, kb % 4, :]
                    # h0: lhsT cols [0:65] -> rows 0-63 data, row 64 denom.
                    # h1: lhsT cols [32:160] -> row 32 denom (the shared ones
                    #     column), rows 64-127 data (partition-aligned with
                    #     at_gp's head-1 half: no cross-partition move).
                    nc.tensor.matmul(
                        pvs[0][0:HD + 1, q0:512], lhsT=vb[:, 0:HD + 1],
                        rhs=pt[:, 0, q0:512],
                        start=(kb == 0), stop=(kb == nkb - 1))
                    nc.tensor.matmul(
                        pvs[1][:, q0:512], lhsT=vb[:, 32:160],
                        rhs=pt[:, 1, q0:512],
                        start=(kb == 0), stop=(kb == nkb - 1))
                for h in range(2):
                    r = 2 * pair + h
                    pvsb = pvsbpool.tile([P, 512], f32, tag="pvsb",
                                         name=f"pvsb_{pair}_{g}_{h}")
                    dr = HD if h == 0 else 32          # denominator row
                    if h == 0:
                        nc.vector.tensor_copy(pvsb[0:HD + 1, :],
                                              pvs[h][0:HD + 1, :])
                    else:
                        nc.vector.tensor_copy(pvsb[:], pvs[h][:])
                    # gather the denominator row (partition move via DMA)
                    nc.sync.dma_start(dn_g[r:r + 1, :], pvsb[dr:dr + 1, :])
                    pvsb_g[r] = pvsb

            def normalize(g, pvsb_g, dn_g):
                rcg = npool.tile([8, 512], f32r, tag="rcg")
                with nc.allow_low_precision(reason="f32r broadcast matmul"):
                    nc.vector.reciprocal(rcg[:], dn_g[:])
                for pair in range(NP):
                    # broadcast 1/denom to the pair's 128 partitions with a
                    # K=8 selection matmul (avoids slow single-partition-
                    # source broadcast DMAs entirely)
                    sxp = pspool.tile([P, 512], f32, tag="ps",
                                      name=f"sx_{g}_{pair}")
                    nc.tensor.matmul(
                        sxp[:], lhsT=es_sb[:, pair, :],
                        rhs=rcg[:], start=True, stop=True)
                    nc.vector.tensor_mul(
                        at_gp[g][pair][0:HD, :],
                        pvsb_g[2 * pair][0:HD, :], sxp[0:HD, :])
                    nc.vector.tensor_mul(
                        at_gp[g][pair][HD:2 * HD, :],
                        pvsb_g[2 * pair + 1][HD:2 * HD, :],
                        sxp[HD:2 * HD, :])

            def outproj(g):
                for qi in range(4):
                    q0 = g * 512 + qi * P
                    yb = ypool.tile([P, D], bf16, tag="yb")
                    for half in range(2):
                        ps = pspool.tile([P, 512], f32, tag="ps",
                                         name=f"yps_{g}_{qi}_{half}")
                        for pair in range(NP):
                            nc.tensor.matmul(
                                ps[:],
                                lhsT=at_gp[g][pair][:, qi * P:(qi + 1) * P],
                                rhs=wp_sb[:, pair,
                                          half * 512:(half + 1) * 512],
                                start=(pair == 0), stop=(pair == NP - 1))
                        nc.vector.tensor_copy(
                            yb[:, half * 512:(half + 1) * 512], ps[:])
                    nc.sync.dma_start(yp_d[q0:q0 + P, :], yb[:])

            # chunk 0 projection (head of the pipeline): K, Q first so the
            # first scores/exps start as early as possible
            for w in (1, 0, 2):
                proj_part(0, w, xt0)
            # prefetch the next x chunk one iteration ahead (xpool bufs=2)
            xts = {1: xpool.tile([P, KD, 512], bf16, tag="xt", name="xt1")}
            nc.sync.dma_start(xts[1][:], xT_d[:, 1])
            # steady state: attention group g = c-1 overlaps projection of
            # chunk c (group g only needs chunks <= g); attention pairs are
            # emitted before proj parts so scores never queue behind proj
            # matmuls in the in-order PE stream
            for c in range(1, NC + 1):
                g = c - 1
                pvsb_g = {}
                dn_g = npool.tile([8, 512], f32, tag="dn", name=f"dn{g}")
                for pair in range(NP):
                    if c < NC and pair < 3:
                        proj_part(c, pair, xts[c])
                    attention(pair, g, pvsb_g, dn_g)
                    if pair == 0 and g >= 1:
                        outproj(g - 1)
                if c + 1 < NC:
                    xts[c + 1] = xpool.tile([P, KD, 512], bf16, tag="xt",
                                            name=f"xt{c + 1}")
                    nc.sync.dma_start(xts[c + 1][:], xT_d[:, c + 1])
                normalize(g, pvsb_g, dn_g)
            outproj(NC - 1)

            if DBG:
                for c in range(NC):
                    nc.sync.dma_start(dbg_qt[:, c], qt_c[c][:])
                    nc.sync.dma_start(dbg_kt[:, c], kt_c[c][:])
                    nc.sync.dma_start(dbg_v[:, c], v_c[c][:])
                    nc.sync.dma_start(dbg_at[:, c], at_g[c][:])

    nc.compile()
    return nc


def _get_nc():
    if "nc" not in _CACHE:
        _CACHE["nc"] = _build_nc()
    return _CACHE["nc"]


def make_in_maps(x, attention_mask, Wq, bq, Wk, bk, Wv, bv, Wp, bp):
    """Host-side sharding: 8 per-core input maps (core = b * 2 + hg)."""
    import ml_dtypes
    bf = ml_dtypes.bfloat16
    x = np.asarray(x, dtype=np.float32)
    scale = np.float32(1.0 / np.sqrt(HD))
    Wq = np.asarray(Wq, np.float32) * scale
    bq = np.asarray(bq, np.float32) * scale
    Wk = np.asarray(Wk, np.float32)
    bk = np.asarray(bk, np.float32)
    Wv = np.asarray(Wv, np.float32)
    Wp = np.asarray(Wp, np.float32)
    mask = np.asarray(attention_mask).astype(np.float32)

    # causal triangle for the diagonal 128x128 square (j-invariant):
    # row p (key), col q: valid iff p <= q.
    pp = np.arange(P)[:, None]
    qq = np.arange(P)[None, :]
    tri = np.where(pp <= qq, 1.0, 0.0).astype(np.float32)
    cm = np.ascontiguousarray(
        np.broadcast_to(tri[:, None, :], (P, 2, P)).astype(ml_dtypes.bfloat16))

    # selection matrix for the reciprocal-broadcast matmul:
    # esel[r, pair, m] = 1 iff r == 2*pair + m//64
    es = np.zeros((8, NP, P), dtype=np.float32)
    for pr in range(NP):
        es[2 * pr, pr, 0:HD] = 1.0
        es[2 * pr + 1, pr, HD:P] = 1.0

    in_maps = []
    for core in range(NCORES):
        b, hg = divmod(core, 2)
        fsl = slice(hg * FPC, (hg + 1) * FPC)
        xT = x[b].T.reshape(KD, P, NC, 512).transpose(1, 2, 0, 3)
        mb = ((mask[b] - 1.0) * np.float32(1e9)).reshape(NB, P).T
        in_maps.append({
            "xT": np.ascontiguousarray(xT.astype(bf)),
            "wq": np.ascontiguousarray(
                Wq[:, fsl].reshape(KD, P, FPC).transpose(1, 0, 2).astype(bf)),
            "wk": np.ascontiguousarray(
                Wk[:, fsl].reshape(KD, P, FPC).transpose(1, 0, 2).astype(bf)),
            "wv": np.ascontiguousarray(
                Wv[:, fsl].reshape(KD, P, FPC).transpose(1, 0, 2).astype(bf)),
            "bq": np.ascontiguousarray(bq[fsl].reshape(NP, P).T),
            "bk": np.ascontiguousarray(bk[fsl].reshape(NP, P).T),
            "wp": np.ascontiguousarray(
                Wp[fsl, :].reshape(NP, P, D).transpose(1, 0, 2).astype(bf)),
            "maskb": np.ascontiguousarray(mb),
            "cmask": cm,
            "esel": es,
        })
    return in_maps


def run(inputs, trace=False, tmpdir=None):
    """Compile (cached) + run on 8 cores. Returns (output, BassKernelResults)."""
    from concourse import bass_utils
    nc = _get_nc()
    in_maps = make_in_maps(**inputs)
    kwargs = {}
    if trace:
        kwargs = dict(trace=True, tmpdir=tmpdir)
    res = bass_utils.run_bass_kernel_spmd(
        nc, in_maps, core_ids=list(range(NCORES)), **kwargs)
    bv = np.asarray(inputs["bv"], np.float64)
    Wp = np.asarray(inputs["Wp"], np.float64)
    bp = np.asarray(inputs["bp"], np.float64)
    const = bv @ Wp + bp  # V-bias contribution (softmax rows sum to 1) + bp
    out = np.empty((B, S, D), dtype=np.float32)
    for b in range(B):
        acc = (res.results[2 * b]["yp"].astype(np.float64)
               + res.results[2 * b + 1]["yp"].astype(np.float64) + const)
        out[b] = acc.astype(np.float32)
    return out, res


def kernel(**inputs) -> np.ndarray:
    out, _ = run(inputs, trace=False)
    return out
